# revision 17
# baseline (speedup 1.0000x reference)
import sys
import numpy as np

sys.path.insert(0, "/opt/trn_rl_repo")

import concourse.bass as bass  # noqa: E402
import concourse.tile as tile  # noqa: E402
import concourse.mybir as mybir  # noqa: E402
from concourse import bacc, bass_utils  # noqa: E402
from contextlib import ExitStack  # noqa: E402

F32 = mybir.dt.float32
BF16 = mybir.dt.bfloat16
I16 = mybir.dt.int16
U8 = mybir.dt.uint8
U16 = mybir.dt.uint16

B = 2048
IN = 2048
F = 2048
SIX = 6
LUT = 64
NCORES = 8
BLOC = B // NCORES      # 256 rows per core
HALF = 1024             # units per half
NIH = SIX * HALF        # 6144 gather slots per half
A = mybir.AluOpType

_CACHED = {}
TRACE = False
LAST = {}


def _build_kernel():
    nc = bacc.Bacc("TRN2", debug=False)

    x0_d = nc.dram_tensor("x0", [BLOC, IN], F32, kind="ExternalInput").ap()
    r_ds = [nc.dram_tensor(f"rj{l}", [BLOC, 2, HALF, SIX], F32,
                           kind="ExternalInput").ap() for l in range(3)]
    lut_ds = [nc.dram_tensor(f"lutT{l}", [LUT, F], F32,
                             kind="ExternalInput").ap() for l in range(3)]
    idx_ds = [nc.dram_tensor(f"idxw{l}", [128, 2 * (NIH // 16)], I16,
                             kind="ExternalInput").ap() for l in range(3)]
    out_d = nc.dram_tensor("out", [BLOC, F], F32, kind="ExternalOutput").ap()

    NW = NIH // 16       # 384 wrapped idx columns per half

    with tile.TileContext(nc) as tc:
        with ExitStack() as ctx:
            cpool = ctx.enter_context(tc.tile_pool(name="const", bufs=1))
            wk = ctx.enter_context(tc.tile_pool(name="wk", bufs=1))

            idxws = [cpool.tile([128, 2 * NW], I16, name=f"idxw{l}")
                     for l in range(3)]
            for l in range(3):
                nc.sync.dma_start(idxws[l][:], idx_ds[l][:])

            # persistent activations (f32), ping-pong across layers
            ybuf = [cpool.tile([128, 2, F], F32, name="yb0"),
                    cpool.tile([128, 2, F], F32, name="yb1")]
            # e codes per btile (bf16, exact ints 0..63)
            ebs = cpool.tile([128, 2, F], BF16, name="ebs")

            for L in range(3):
                ysrc = ybuf[(L - 1) % 2]
                ydst = ybuf[L % 2]

                for bt in range(2):
                    if L == 0:
                        nc.sync.dma_start(
                            ysrc[:, bt], x0_d[bt * 128:(bt + 1) * 128, :])

                    for h in range(2):
                        xg = wk.tile([128, NIH], F32, tag="xg",
                                     name="xg", bufs=2)
                        nc.gpsimd.ap_gather(
                            xg[:], ysrc[:, bt],
                            idxws[L][:, h * NW:(h + 1) * NW],
                            channels=128, num_elems=F, d=1, num_idxs=NIH)
                        rq = wk.tile([128, NIH], F32, tag="rq",
                                     name="rq", bufs=2)
                        nc.sync.dma_start(
                            rq[:], r_ds[L][bt * 128:(bt + 1) * 128, h])

                        bits = wk.tile([128, NIH], BF16, tag="bits",
                                       name="bits", bufs=2)
                        nc.vector.tensor_tensor(bits[:], xg[:], rq[:],
                                                A.is_ge)
                        # horner pack: e = ((((b0*2+b1)*2+b2)...)*2+b5)
                        b3 = bits[:].rearrange("p (f j) -> p f j", j=SIX)
                        esl = ebs[:, bt, h * HALF:(h + 1) * HALF]
                        nc.vector.scalar_tensor_tensor(
                            esl, b3[:, :, 0], 2.0, b3[:, :, 1],
                            A.mult, A.add)
                        for j in range(2, SIX):
                            nc.vector.scalar_tensor_tensor(
                                esl, esl, 2.0, b3[:, :, j], A.mult, A.add)

                        # 64-way select for this f-half immediately: the
                        # h=0 select runs inside the h=1 gather's window,
                        # so ydst[:, bt] completes earlier and the next
                        # layer's gathers start sooner
                        hs = slice(h * HALF, (h + 1) * HALF)
                        for k in range(LUT):
                            lutb = wk.tile([128, HALF], F32, tag="lutb",
                                           name="lutb", bufs=4)
                            eng = nc.sync if k % 2 == 0 else nc.scalar
                            eng.dma_start(
                                lutb[:],
                                lut_ds[L][k:k + 1, hs]
                                .to_broadcast([128, HALF]))
                            m = wk.tile([128, HALF], U16, tag="mask",
                                        name="mask", bufs=2)
                            nc.vector.tensor_scalar(
                                m[:], esl, float(k), None, A.is_equal)
                            nc.vector.copy_predicated(
                                ydst[:, bt, hs], m[:], lutb[:])

                if L == 2:
                    for bt in range(2):
                        nc.sync.dma_start(
                            out_d[bt * 128:(bt + 1) * 128, :], ydst[:, bt])

    nc.compile()
    return nc


def _brev6(k):
    r = 0
    for i in range(6):
        r |= ((k >> i) & 1) << (5 - i)
    return r


def _prep_host(lut1, lut2, lut3, connect_1, connect_2, connect_3):
    def sig(x):
        return (1.0 / (1.0 + np.exp(-np.asarray(x, np.float64)))).astype(
            np.float32)

    # horner gives e = sum_j bit_j * 2^(5-j) (MSB-first) -> bit-reversed cols
    brev = np.array([_brev6(k) for k in range(LUT)])
    lutTs = [np.ascontiguousarray(sig(lut1)[:, brev].T),
             np.ascontiguousarray(sig(lut2)[:, brev].T),
             np.ascontiguousarray(
                 np.asarray(lut3, np.float32)[:, brev].T)]  # [64, F]

    # ap_gather wrapped idx, slot-major (j fastest) within each half
    idxws = []
    for c in (connect_1, connect_2, connect_3):
        cc = np.asarray(c, np.int64)                       # [F, SIX]
        halves = []
        for h in range(2):
            flat = np.ascontiguousarray(
                cc[h * HALF:(h + 1) * HALF, :]).reshape(NIH)
            wrapped = flat.reshape(NIH // 16, 16).T        # [16, 384]
            halves.append(np.tile(wrapped, (8, 1)))        # [128, 384]
        idxws.append(np.ascontiguousarray(
            np.concatenate(halves, axis=1)).astype(np.int16))
    return lutTs, idxws


def _prep_core(inputs, rs, c):
    sl = slice(c * BLOC, (c + 1) * BLOC)
    x0 = ((np.asarray(inputs[sl], np.float32) + np.float32(1.0))
          * np.float32(0.5)).astype(np.float32)
    rjs = []
    for r in rs:
        rr = np.asarray(r[sl], np.float32).reshape(BLOC, 2, HALF, SIX)
        rjs.append(np.ascontiguousarray(rr))
    return x0, rjs


def kernel(inputs, r1, r2, r3, lut1, lut2, lut3,
           connect_1, connect_2, connect_3):
    inputs = np.asarray(inputs, np.float32)
    lutTs, idxws = _prep_host(lut1, lut2, lut3,
                              connect_1, connect_2, connect_3)

    if "nc" not in _CACHED:
        _CACHED["nc"] = _build_kernel()
    nc = _CACHED["nc"]

    in_maps = []
    for c in range(NCORES):
        x0, rjs = _prep_core(inputs, (r1, r2, r3), c)
        m = {"x0": x0}
        for l in range(3):
            m[f"rj{l}"] = rjs[l]
            m[f"lutT{l}"] = lutTs[l]
            m[f"idxw{l}"] = idxws[l]
        in_maps.append(m)

    if TRACE:
        import tempfile
        tmpdir = tempfile.mkdtemp(prefix="bass_trace_")
        res = bass_utils.run_bass_kernel_spmd(
            nc, in_maps, core_ids=list(range(NCORES)), trace=True,
            tmpdir=tmpdir)
        LAST["exec_ns"] = res.exec_time_ns
        LAST["trace_dir"] = tmpdir
        LAST["res"] = res
    else:
        res = bass_utils.run_bass_kernel_spmd(
            nc, in_maps, core_ids=list(range(NCORES)))
    out = np.concatenate([res.results[c]["out"] for c in range(NCORES)],
                         axis=0)
    return out.astype(np.float32)
